# revision 10
# baseline (speedup 1.0000x reference)
"""Trainium2 Bass kernel for nn_Attention_4183298146960.

GQA causal attention layer: B=2, S=2048, HIDDEN=2048, 16 q heads / 4 kv heads,
head_dim=128, RoPE (interleaved pairs), causal softmax, output projection.

Sharding (8 cores, SPMD-uniform program, batch-split tensor parallel):
  core c owns batch b = c//4 and q heads {4g..4g+3}, kv head g, where g = c%4.
  QKV + RoPE + attention fully local (no duplicated kv compute across cores).
  The output projection needs all 16 heads' features, so per-core attention
  outputs [512 feat, 512 tok] are AllGathered within the 4-core batch group
  ([[0,1,2,3],[4,5,6,7]]); each core then computes its 512 output columns.

Schedule: attention chunk qt runs immediately after t-tile qt's QKV (causal:
  it only needs k/v tiles <= qt), so the first AllGather triggers ~35us into
  the kernel and the serialized collective chain hides under remaining
  QKV/attention/W_o compute.  W_o for chunk qt is emitted after attention
  chunk qt+1 so the PE stream never waits on a fresh gather.

Layouts on device (partition dim first):
  feature-major qT/kT [head_dim, tokens] for scores; token-major v
  [tokens, head_dim] for PV; scores computed transposed [k, q] so softmax
  needs no max-subtraction (scores are O(+-10), exp is fp32-safe) and the
  denominator is a ones-matrix matmul producing the broadcast denominator
  directly; probabilities stay unnormalized until after PV.
  RoPE head dims are permuted [even | odd] via host-side W row permutation
  so the rotation is a 64-partition swap (SBUF->SBUF DMA) + DVE ops.
"""

import numpy as np
import ml_dtypes

import concourse.bass as bass
import concourse.mybir as mybir
import concourse.tile as tile
from concourse import bacc
from concourse.bass_utils import run_bass_kernel_spmd

BF16 = ml_dtypes.bfloat16

HEADS = 16
KV_HEADS = 4
HIDDEN = 2048
HD = 128
S = 2048
B = 2
HT = HIDDEN // 128             # 16 hidden tiles
NH = 4                         # local q heads per core
SCALE = 1.0 / float(np.sqrt(HD))
RG = [[0, 1, 2, 3], [4, 5, 6, 7]]

_COMPILED = None


def _build():
    dt = mybir.dt
    nc = bacc.Bacc("TRN2", target_bir_lowering=False, debug=False, num_devices=8)

    xT = nc.dram_tensor("xT", [128, HT, S], dt.bfloat16, kind="ExternalInput")
    wqk = nc.dram_tensor("wqk", [128, HT, 640], dt.bfloat16, kind="ExternalInput")
    wv = nc.dram_tensor("wv", [128, HT, 128], dt.bfloat16, kind="ExternalInput")
    wo = nc.dram_tensor("wo", [128, HT, 512], dt.bfloat16, kind="ExternalInput")
    cc = nc.dram_tensor("cc", [128, S], dt.bfloat16, kind="ExternalInput")
    ss = nc.dram_tensor("ss", [128, S], dt.bfloat16, kind="ExternalInput")
    msk = nc.dram_tensor("msk", [128, 4 * 512], dt.bfloat16, kind="ExternalInput")
    ones128 = nc.dram_tensor("ones128", [128, 128], dt.bfloat16, kind="ExternalInput")
    outT = nc.dram_tensor("outT", [512, S], dt.float32, kind="ExternalOutput")

    mult = mybir.AluOpType.mult
    add = mybir.AluOpType.add
    Exp = mybir.ActivationFunctionType.Exp

    with tile.TileContext(nc) as tc:
        with (
            tc.tile_pool(name="const", bufs=1) as constp,
            tc.tile_pool(name="dram", bufs=1, space="DRAM") as dram,
            tc.tile_pool(name="xp", bufs=2) as xp,
            tc.tile_pool(name="rp", bufs=4) as rp,
            tc.tile_pool(name="probs", bufs=6) as probs,
            tc.tile_pool(name="smallp", bufs=4) as smallp,
            tc.tile_pool(name="ap", bufs=3) as apool,
            tc.tile_pool(name="wosb", bufs=2) as wosb,
            tc.tile_pool(name="outp", bufs=2) as outp,
            tc.tile_pool(name="qkps", bufs=2, space="PSUM") as qkps,
            tc.tile_pool(name="vps", bufs=1, space="PSUM") as vps,
            tc.tile_pool(name="spool", bufs=2, space="PSUM") as spool,
            tc.tile_pool(name="pvp", bufs=1, space="PSUM") as pvp,
        ):
            qcat = constp.tile([128, NH * S], dt.bfloat16)  # 4 local q heads
            kT = constp.tile([128, S], dt.bfloat16)
            vsb = constp.tile([128, S], dt.bfloat16)        # token-major v tiles
            wo_sb = constp.tile([128, HT, 512], dt.bfloat16)
            msk_sb = constp.tile([128, 4 * 512], dt.bfloat16)
            ones_sb = constp.tile([128, 128], dt.bfloat16)
            wqk_sb = constp.tile([128, HT, 640], dt.bfloat16)
            wv_sb = constp.tile([128, HT, 128], dt.bfloat16)
            cc_sb = constp.tile([128, S], dt.bfloat16)
            ss_sb = constp.tile([128, S], dt.bfloat16)

            for hq in range(4):   # split: first MMs start after 1/4 arrives
                nc.sync.dma_start(
                    wqk_sb[:, hq * 4:(hq + 1) * 4, :], wqk[:, hq * 4:(hq + 1) * 4, :]
                )
            nc.sync.dma_start(wv_sb[:], wv[:])
            nc.sync.dma_start(cc_sb[:], cc[:])
            nc.sync.dma_start(ss_sb[:], ss[:])
            nc.sync.dma_start(msk_sb[:], msk[:])
            nc.sync.dma_start(ones_sb[:], ones128[:])
            nc.sync.dma_start(wo_sb[:], wo[:])

            def emit_att(qt, j):
                """Attention for 512-token chunk qt (k/v tiles <= 4qt+3 ready)."""
                attn_chunk = dram.tile([512, 512], dt.bfloat16, name=f"attnc{j}")
                kts = 4 * qt + 4
                for hp in range(2):            # head pairs (2hp, 2hp+1)
                    ps_pv = [pvp.tile([128, 512], dt.float32, name=f"pv{hh}")
                             for hh in range(2)]
                    acc = [smallp.tile([128, 512], dt.bfloat16, name=f"acc{hh}", bufs=2)
                           for hh in range(2)]
                    for kt in range(kts):
                        r = kt - 4 * qt
                        prob2 = []
                        for hh in range(2):
                            h = 2 * hp + hh
                            ps_s = spool.tile([128, 512], dt.float32)
                            nc.tensor.matmul(
                                ps_s[:],
                                lhsT=kT[:, kt * 128:(kt + 1) * 128],
                                rhs=qcat[:, h * S + qt * 512: h * S + (qt + 1) * 512],
                                start=True,
                                stop=True,
                            )
                            prob = probs.tile([128, 512], dt.bfloat16)
                            if r >= 0:
                                stg = probs.tile([128, 512], dt.bfloat16, name="stg")
                                nc.scalar.activation(stg[:], ps_s[:], Exp, scale=SCALE)
                                nc.vector.tensor_tensor(
                                    prob[:], stg[:], msk_sb[:, r * 512:(r + 1) * 512], mult
                                )
                            else:
                                nc.scalar.activation(prob[:], ps_s[:], Exp, scale=SCALE)
                            prob2.append(prob)
                        for hh in range(2):
                            nc.tensor.matmul(
                                ps_pv[hh][:],
                                lhsT=vsb[:, kt * 128:(kt + 1) * 128],
                                rhs=prob2[hh][:],
                                start=(kt == 0), stop=(kt == kts - 1),
                            )
                        for hh in range(2):
                            # denominator partial sums on DVE (sum over k-tiles)
                            if kt == 0:
                                nc.vector.tensor_copy(acc[hh][:], prob2[hh][:])
                            else:
                                nc.vector.tensor_add(acc[hh][:], acc[hh][:], prob2[hh][:])
                    for hh in range(2):
                        # partition-reduce + broadcast denominators in one matmul
                        ps_den = spool.tile([128, 512], dt.float32, name="den", bufs=1)
                        nc.tensor.matmul(
                            ps_den[:], lhsT=ones_sb[:], rhs=acc[hh][:],
                            start=True, stop=True,
                        )
                        den_sb = smallp.tile([128, 512], dt.float32)
                        nc.vector.reciprocal_approx_fast(den_sb[:], ps_den[:])
                        attn_sb = apool.tile([128, 512], dt.bfloat16)
                        nc.vector.tensor_tensor(attn_sb[:], ps_pv[hh][:], den_sb[:], mult)
                        lh = 2 * hp + hh
                        nc.sync.dma_start(
                            attn_chunk[lh * 128:(lh + 1) * 128, :], attn_sb[:]
                        )
                ag_out = dram.tile([HT, 128, 512], dt.bfloat16, name=f"agout{j}")
                nc.gpsimd.collective_compute(
                    "AllGather", mybir.AluOpType.bypass, replica_groups=RG,
                    ins=[attn_chunk.opt()], outs=[ag_out.opt()],
                )
                return ag_out

            def emit_wo(qt, ag_out):
                # W_o for one chunk's tokens; deferred one chunk so the PE
                # stream never waits on a fresh gather.
                asb = wosb.tile([128, HT, 512], dt.bfloat16)
                for dtt in range(HT):
                    nc.sync.dma_start(asb[:, dtt, :], ag_out[dtt, :, :])
                for ct in range(4):
                    ps_o = qkps.tile([128, 512], dt.float32, name="qk")
                    for dtt in range(HT):
                        nc.tensor.matmul(
                            ps_o[:],
                            lhsT=wo_sb[:, dtt, ct * 128:(ct + 1) * 128],
                            rhs=asb[:, dtt, :],
                            start=(dtt == 0), stop=(dtt == HT - 1),
                        )
                    o_sb = outp.tile([128, 512], dt.float32)
                    nc.vector.tensor_copy(o_sb[:], ps_o[:])
                    nc.sync.dma_start(
                        outT[ct * 128:(ct + 1) * 128, qt * 512:(qt + 1) * 512],
                        o_sb[:],
                    )

            pending_wo = []
            for tt in range(4):
                x_sb = xp.tile([128, HT, 512], dt.bfloat16)
                for hq in range(4):  # split so MMs can start on early ht tiles
                    nc.sync.dma_start(
                        x_sb[:, hq * 4:(hq + 1) * 4, :],
                        xT[:, hq * 4:(hq + 1) * 4, tt * 512:(tt + 1) * 512],
                    )
                tsl = bass.ts(tt, 512)
                for ft in range(5):
                    ps = qkps.tile([128, 512], dt.float32, name="qk")
                    for ht in range(HT):
                        nc.tensor.matmul(
                            ps[:],
                            lhsT=wqk_sb[:, ht, ft * 128:(ft + 1) * 128],
                            rhs=x_sb[:, ht, :],
                            start=(ht == 0),
                            stop=(ht == HT - 1),
                        )
                    sbq = rp.tile([128, 512], dt.bfloat16)
                    nc.scalar.copy(sbq[:], ps[:])
                    tmp = rp.tile([128, 512], dt.bfloat16)
                    nc.gpsimd.dma_start(tmp[0:64, :], sbq[64:128, :])
                    nc.gpsimd.dma_start(tmp[64:128, :], sbq[0:64, :])
                    qcc = rp.tile([128, 512], dt.bfloat16)
                    nc.vector.tensor_tensor(qcc[:], sbq[:], cc_sb[:, tsl], mult)
                    qss = rp.tile([128, 512], dt.bfloat16)
                    nc.vector.tensor_tensor(qss[:], tmp[:], ss_sb[:, tsl], mult)
                    if ft < 4:
                        dst = qcat[:, ft * S + tt * 512: ft * S + (tt + 1) * 512]
                    else:
                        dst = kT[:, tsl]
                    nc.vector.tensor_tensor(dst, qcc[:], qss[:], add)
                # token-major v for this t-tile
                for st in range(4):
                    psv = vps.tile([128, 128], dt.float32)
                    for ht in range(HT):
                        nc.tensor.matmul(
                            psv[:],
                            lhsT=x_sb[:, ht, st * 128:(st + 1) * 128],
                            rhs=wv_sb[:, ht, :],
                            start=(ht == 0),
                            stop=(ht == HT - 1),
                        )
                    t128 = tt * 4 + st
                    nc.scalar.copy(vsb[:, t128 * 128:(t128 + 1) * 128], psv[:])
                # attention for this chunk, then W_o for the previous one
                ag_out = emit_att(tt, tt)
                pending_wo.append((tt, ag_out))
                if len(pending_wo) > 1:
                    emit_wo(*pending_wo.pop(0))
            for w in pending_wo:
                emit_wo(*w)
    nc.compile()
    return nc


# host-side input prep ------------------------------------------------------

_PERM = np.concatenate([np.arange(0, HD, 2), np.arange(1, HD, 2)])


def _rope_tables():
    freq = 1.0 / (10000.0 ** (np.arange(0, HD, 2, dtype=np.float64) / HD))
    pos = np.arange(S, dtype=np.float64)
    ang = np.outer(pos, freq)                       # [S, 64]
    cos = np.cos(ang).T.astype(np.float32)          # [64, S]
    sin = np.sin(ang).T.astype(np.float32)
    cc1 = np.concatenate([cos, cos], 0)             # [128, S]
    ss1 = np.concatenate([-sin, sin], 0)            # [128, S]
    return cc1.astype(BF16), ss1.astype(BF16)


def _prep_inputs(x, W_qkv, W_o):
    x = np.asarray(x, dtype=np.float32)
    W_qkv = np.asarray(W_qkv, dtype=np.float32)
    W_o = np.asarray(W_o, dtype=np.float32)

    xTb = []
    for b in range(B):
        xTb.append(np.ascontiguousarray(
            x[b].T.reshape(HT, 128, S).transpose(1, 0, 2)
        ).astype(BF16))                              # [128, HT, 2048]

    cc, ss = _rope_tables()

    mask = np.zeros((128, 4 * 512), dtype=np.float32)
    ii = np.arange(128)[:, None]
    jj = np.arange(512)[None, :]
    for r in range(4):
        mask[:, r * 512:(r + 1) * 512] = (jj >= ii + 128 * r)
    mask = mask.astype(BF16)

    ones128 = np.ones((128, 128), dtype=np.float32).astype(BF16)

    in_maps = []
    for c in range(8):
        b, g = c // 4, c % 4
        qr = W_qkv[512 * g: 512 * (g + 1)]           # rows of q heads 4g..4g+3
        qr = qr.reshape(NH, HD, HIDDEN)[:, _PERM, :].reshape(512, HIDDEN)
        kr = W_qkv[HIDDEN + 128 * g: HIDDEN + 128 * (g + 1)][_PERM, :]
        vr = W_qkv[HIDDEN + 512 + 128 * g: HIDDEN + 512 + 128 * (g + 1)]
        wqkT = np.ascontiguousarray(
            np.concatenate([qr, kr], 0).T.reshape(HT, 128, 640).transpose(1, 0, 2)
        ).astype(BF16)                               # [128, HT, 640]
        wvT = np.ascontiguousarray(
            vr.T.reshape(HT, 128, 128).transpose(1, 0, 2)
        ).astype(BF16)
        woT = np.ascontiguousarray(
            W_o[512 * g: 512 * (g + 1)].T.reshape(HT, 128, 512).transpose(1, 0, 2)
        ).astype(BF16)
        in_maps.append({
            "xT": xTb[b], "wqk": wqkT, "wv": wvT, "wo": woT,
            "cc": cc, "ss": ss, "msk": mask, "ones128": ones128,
        })
    return in_maps


def kernel(x, W_qkv, W_o):
    global _COMPILED
    if _COMPILED is None:
        _COMPILED = _build()
    nc = _COMPILED
    in_maps = _prep_inputs(x, W_qkv, W_o)
    res = run_bass_kernel_spmd(nc, in_maps, list(range(8)))
    out = np.empty((B, S, HIDDEN), dtype=np.float32)
    for c in range(8):
        b, g = c // 4, c % 4
        oT = res.results[c]["outT"]                  # [512, 2048]
        out[b, :, 512 * g: 512 * (g + 1)] = oT.T
    return out


# revision 17
# speedup vs baseline: 1.0999x; 1.0999x over previous
"""Trainium2 Bass kernel for nn_Attention_4183298146960.

GQA causal attention layer: B=2, S=2048, HIDDEN=2048, 16 q heads / 4 kv heads,
head_dim=128, RoPE (interleaved pairs), causal softmax, output projection.

Sharding (8 cores, SPMD-uniform program, batch-split tensor parallel):
  core c owns batch b = c//4 and q heads {4g..4g+3}, kv head g, where g = c%4.
  QKV + RoPE + attention fully local (no duplicated kv compute across cores).
  The output projection needs all 16 heads' features, so per-core attention
  outputs [512 feat, 512 tok] are AllGathered within the 4-core batch group
  ([[0,1,2,3],[4,5,6,7]]); each core then computes its 512 output columns.

Schedule: attention chunk qt runs immediately after t-tile qt's QKV (causal:
  it only needs k/v tiles <= qt), so the first AllGather triggers ~35us into
  the kernel and the serialized collective chain hides under remaining
  QKV/attention/W_o compute.  W_o for chunk qt is emitted after attention
  chunk qt+1 so the PE stream never waits on a fresh gather.

Layouts on device (partition dim first):
  feature-major qT/kT [head_dim, tokens] for scores; token-major v
  [tokens, head_dim] for PV; scores computed transposed [k, q] so softmax
  needs no max-subtraction (scores are O(+-10), exp is fp32-safe) and the
  denominator is a ones-matrix matmul producing the broadcast denominator
  directly; probabilities stay unnormalized until after PV.
  RoPE head dims are permuted [even | odd] via host-side W row permutation
  so the rotation is a 64-partition swap (SBUF->SBUF DMA) + DVE ops.
"""

import numpy as np
import ml_dtypes

import concourse.bass as bass
import concourse.mybir as mybir
import concourse.tile as tile
from concourse import bacc
from concourse.bass_utils import run_bass_kernel_spmd

BF16 = ml_dtypes.bfloat16

HEADS = 16
KV_HEADS = 4
HIDDEN = 2048
HD = 128
S = 2048
B = 2
HT = HIDDEN // 128             # 16 hidden tiles
NH = 4                         # local q heads per core
SCALE = 1.0 / float(np.sqrt(HD))
RG = [[0, 1, 2, 3], [4, 5, 6, 7]]

_COMPILED = None


def _build():
    dt = mybir.dt
    nc = bacc.Bacc("TRN2", target_bir_lowering=False, debug=False, num_devices=8)

    xT = nc.dram_tensor("xT", [128, HT, S], dt.bfloat16, kind="ExternalInput")
    wqk = nc.dram_tensor("wqk", [128, HT, 640], dt.bfloat16, kind="ExternalInput")
    wv = nc.dram_tensor("wv", [128, HT, 128], dt.bfloat16, kind="ExternalInput")
    wo = nc.dram_tensor("wo", [128, HT, 512], dt.bfloat16, kind="ExternalInput")
    cc = nc.dram_tensor("cc", [128, S], dt.bfloat16, kind="ExternalInput")
    ss = nc.dram_tensor("ss", [128, S], dt.bfloat16, kind="ExternalInput")
    msk = nc.dram_tensor("msk", [128, 4 * 512], dt.bfloat16, kind="ExternalInput")
    ones128 = nc.dram_tensor("ones128", [128, 128], dt.bfloat16, kind="ExternalInput")
    outT = nc.dram_tensor("outT", [512, S], dt.float32, kind="ExternalOutput")

    mult = mybir.AluOpType.mult
    add = mybir.AluOpType.add
    Exp = mybir.ActivationFunctionType.Exp

    with tile.TileContext(nc) as tc:
        with (
            tc.tile_pool(name="const", bufs=1) as constp,
            tc.tile_pool(name="dram", bufs=1, space="DRAM") as dram,
            tc.tile_pool(name="xp", bufs=2) as xp,
            tc.tile_pool(name="rp", bufs=4) as rp,
            tc.tile_pool(name="probs", bufs=6) as probs,
            tc.tile_pool(name="smallp", bufs=4) as smallp,
            tc.tile_pool(name="ap", bufs=3) as apool,
            tc.tile_pool(name="wosb", bufs=2) as wosb,
            tc.tile_pool(name="outp", bufs=2) as outp,
            tc.tile_pool(name="qkps", bufs=2, space="PSUM") as qkps,
            tc.tile_pool(name="spool", bufs=4, space="PSUM") as spool,
            tc.tile_pool(name="pvp", bufs=1, space="PSUM") as pvp,
        ):
            qcat = constp.tile([128, NH * S], dt.bfloat16)  # 4 local q heads
            kT = constp.tile([128, S], dt.bfloat16)
            vsb = constp.tile([128, S], dt.bfloat16)        # token-major v tiles
            wo_sb = constp.tile([128, HT, 512], dt.bfloat16)
            msk_sb = constp.tile([128, 4 * 512], dt.bfloat16)
            ones_sb = constp.tile([128, 128], dt.bfloat16)
            wqk_sb = constp.tile([128, HT, 640], dt.bfloat16)
            wv_sb = constp.tile([128, HT, 128], dt.bfloat16)
            cc_sb = constp.tile([128, S], dt.bfloat16)
            ss_sb = constp.tile([128, S], dt.bfloat16)

            # first quarter of the qkv weights only — x(t0) must follow quickly
            nc.sync.dma_start(wqk_sb[:, 0:4, :], wqk[:, 0:4, :])

            def emit_att(qt, j):
                """Attention for 512-token chunk qt (k/v tiles <= 4qt+3 ready)."""
                attn_chunk = dram.tile([512, 512], dt.bfloat16, name=f"attnc{j}")
                kts = 4 * qt + 4
                for hp in range(2):            # head pairs (2hp, 2hp+1)
                    ps_pv = [pvp.tile([128, 512], dt.float32, name=f"pv{hh}")
                             for hh in range(2)]
                    acc = [smallp.tile([128, 512], dt.bfloat16, name=f"acc{hh}", bufs=2)
                           for hh in range(2)]
                    for kt in range(kts):
                        r = kt - 4 * qt
                        prob2 = []
                        for hh in range(2):
                            h = 2 * hp + hh
                            ps_s = spool.tile([128, 512], dt.float32)
                            nc.tensor.matmul(
                                ps_s[:],
                                lhsT=kT[:, kt * 128:(kt + 1) * 128],
                                rhs=qcat[:, h * S + qt * 512: h * S + (qt + 1) * 512],
                                start=True,
                                stop=True,
                            )
                            prob = probs.tile([128, 512], dt.bfloat16)
                            if r >= 0:
                                stg = probs.tile([128, 512], dt.bfloat16, name="stg")
                                nc.scalar.activation(stg[:], ps_s[:], Exp, scale=SCALE)
                                nc.vector.tensor_tensor(
                                    prob[:], stg[:], msk_sb[:, r * 512:(r + 1) * 512], mult
                                )
                            else:
                                nc.scalar.activation(prob[:], ps_s[:], Exp, scale=SCALE)
                            prob2.append(prob)
                        for hh in range(2):
                            nc.tensor.matmul(
                                ps_pv[hh][:],
                                lhsT=vsb[:, kt * 128:(kt + 1) * 128],
                                rhs=prob2[hh][:],
                                start=(kt == 0), stop=(kt == kts - 1),
                            )
                        for hh in range(2):
                            # denominator partial sums on DVE (sum over k-tiles)
                            if kt == 0:
                                nc.vector.tensor_copy(acc[hh][:], prob2[hh][:])
                            else:
                                nc.vector.tensor_add(acc[hh][:], acc[hh][:], prob2[hh][:])
                    for hh in range(2):
                        # partition-reduce + broadcast denominators in one matmul
                        ps_den = qkps.tile([128, 512], dt.float32, name="qk")
                        nc.tensor.matmul(
                            ps_den[:], lhsT=ones_sb[:], rhs=acc[hh][:],
                            start=True, stop=True,
                        )
                        den_sb = smallp.tile([128, 512], dt.float32)
                        nc.vector.reciprocal_approx_fast(den_sb[:], ps_den[:])
                        attn_sb = apool.tile([128, 512], dt.bfloat16)
                        nc.vector.tensor_tensor(attn_sb[:], ps_pv[hh][:], den_sb[:], mult)
                        lh = 2 * hp + hh
                        nc.sync.dma_start(
                            attn_chunk[lh * 128:(lh + 1) * 128, :], attn_sb[:]
                        )
                ag_out = dram.tile([HT, 128, 512], dt.bfloat16, name=f"agout{j}")
                nc.gpsimd.collective_compute(
                    "AllGather", mybir.AluOpType.bypass, replica_groups=RG,
                    ins=[attn_chunk.opt()], outs=[ag_out.opt()],
                )
                return ag_out

            def emit_wo(qt, ag_out):
                # W_o for one chunk's tokens; deferred one chunk so the PE
                # stream never waits on a fresh gather.
                asb = wosb.tile([128, HT, 512], dt.bfloat16)
                for dtt in range(HT):
                    nc.sync.dma_start(asb[:, dtt, :], ag_out[dtt, :, :])
                for ct in range(4):
                    ps_o = qkps.tile([128, 512], dt.float32, name="qk")
                    for dtt in range(HT):
                        nc.tensor.matmul(
                            ps_o[:],
                            lhsT=wo_sb[:, dtt, ct * 128:(ct + 1) * 128],
                            rhs=asb[:, dtt, :],
                            start=(dtt == 0), stop=(dtt == HT - 1),
                        )
                    o_sb = outp.tile([128, 512], dt.float32)
                    nc.vector.tensor_copy(o_sb[:], ps_o[:])
                    nc.sync.dma_start(
                        outT[ct * 128:(ct + 1) * 128, qt * 512:(qt + 1) * 512],
                        o_sb[:],
                    )

            pending_wo = []
            for tt in range(4):
                x_sb = xp.tile([128, HT, 512], dt.bfloat16)
                for hq in range(4):  # split so MMs can start on early ht tiles
                    nc.sync.dma_start(
                        x_sb[:, hq * 4:(hq + 1) * 4, :],
                        xT[:, hq * 4:(hq + 1) * 4, tt * 512:(tt + 1) * 512],
                    )
                if tt == 0:
                    # remaining weights/tables, behind the x(t0) stream
                    nc.sync.dma_start(cc_sb[:], cc[:])
                    nc.sync.dma_start(ss_sb[:], ss[:])
                    for hq in range(1, 4):
                        nc.sync.dma_start(
                            wqk_sb[:, hq * 4:(hq + 1) * 4, :],
                            wqk[:, hq * 4:(hq + 1) * 4, :],
                        )
                    nc.sync.dma_start(wv_sb[:], wv[:])
                    nc.sync.dma_start(msk_sb[:], msk[:])
                    nc.sync.dma_start(ones_sb[:], ones128[:])
                    nc.sync.dma_start(wo_sb[:], wo[:])
                tsl = bass.ts(tt, 512)
                for ft in range(5):
                    ps = qkps.tile([128, 512], dt.float32, name="qk")
                    for ht in range(HT):
                        nc.tensor.matmul(
                            ps[:],
                            lhsT=wqk_sb[:, ht, ft * 128:(ft + 1) * 128],
                            rhs=x_sb[:, ht, :],
                            start=(ht == 0),
                            stop=(ht == HT - 1),
                        )
                    sbq = rp.tile([128, 512], dt.bfloat16)
                    nc.scalar.copy(sbq[:], ps[:])
                    tmp = rp.tile([128, 512], dt.bfloat16)
                    # sync queue, NOT gpsimd: the collective trigger+wait
                    # occupies the gpsimd queue and would gate the rope here
                    nc.sync.dma_start(tmp[0:64, :], sbq[64:128, :])
                    nc.sync.dma_start(tmp[64:128, :], sbq[0:64, :])
                    qcc = rp.tile([128, 512], dt.bfloat16)
                    nc.vector.tensor_tensor(qcc[:], sbq[:], cc_sb[:, tsl], mult)
                    qss = rp.tile([128, 512], dt.bfloat16)
                    nc.vector.tensor_tensor(qss[:], tmp[:], ss_sb[:, tsl], mult)
                    if ft < 4:
                        dst = qcat[:, ft * S + tt * 512: ft * S + (tt + 1) * 512]
                    else:
                        dst = kT[:, tsl]
                    nc.vector.tensor_tensor(dst, qcc[:], qss[:], add)
                # token-major v for this t-tile (psum shares the qk rotation)
                for st in range(4):
                    psv = qkps.tile([128, 512], dt.float32, name="qk")
                    for ht in range(HT):
                        nc.tensor.matmul(
                            psv[:, 0:128],
                            lhsT=x_sb[:, ht, st * 128:(st + 1) * 128],
                            rhs=wv_sb[:, ht, :],
                            start=(ht == 0),
                            stop=(ht == HT - 1),
                        )
                    t128 = tt * 4 + st
                    nc.scalar.copy(vsb[:, t128 * 128:(t128 + 1) * 128], psv[:, 0:128])
                # attention for this chunk, then W_o for the previous one
                ag_out = emit_att(tt, tt)
                pending_wo.append((tt, ag_out))
                if len(pending_wo) > 2:
                    emit_wo(*pending_wo.pop(0))
            for w in pending_wo:
                emit_wo(*w)
    nc.compile()
    return nc


# host-side input prep ------------------------------------------------------

_PERM = np.concatenate([np.arange(0, HD, 2), np.arange(1, HD, 2)])


def _rope_tables():
    freq = 1.0 / (10000.0 ** (np.arange(0, HD, 2, dtype=np.float64) / HD))
    pos = np.arange(S, dtype=np.float64)
    ang = np.outer(pos, freq)                       # [S, 64]
    cos = np.cos(ang).T.astype(np.float32)          # [64, S]
    sin = np.sin(ang).T.astype(np.float32)
    cc1 = np.concatenate([cos, cos], 0)             # [128, S]
    ss1 = np.concatenate([-sin, sin], 0)            # [128, S]
    return cc1.astype(BF16), ss1.astype(BF16)


def _prep_inputs(x, W_qkv, W_o):
    x = np.asarray(x, dtype=np.float32)
    W_qkv = np.asarray(W_qkv, dtype=np.float32)
    W_o = np.asarray(W_o, dtype=np.float32)

    xTb = []
    for b in range(B):
        xTb.append(np.ascontiguousarray(
            x[b].T.reshape(HT, 128, S).transpose(1, 0, 2)
        ).astype(BF16))                              # [128, HT, 2048]

    cc, ss = _rope_tables()

    mask = np.zeros((128, 4 * 512), dtype=np.float32)
    ii = np.arange(128)[:, None]
    jj = np.arange(512)[None, :]
    for r in range(4):
        mask[:, r * 512:(r + 1) * 512] = (jj >= ii + 128 * r)
    mask = mask.astype(BF16)

    ones128 = np.ones((128, 128), dtype=np.float32).astype(BF16)

    in_maps = []
    for c in range(8):
        b, g = c // 4, c % 4
        qr = W_qkv[512 * g: 512 * (g + 1)]           # rows of q heads 4g..4g+3
        qr = qr.reshape(NH, HD, HIDDEN)[:, _PERM, :].reshape(512, HIDDEN)
        kr = W_qkv[HIDDEN + 128 * g: HIDDEN + 128 * (g + 1)][_PERM, :]
        vr = W_qkv[HIDDEN + 512 + 128 * g: HIDDEN + 512 + 128 * (g + 1)]
        wqkT = np.ascontiguousarray(
            np.concatenate([qr, kr], 0).T.reshape(HT, 128, 640).transpose(1, 0, 2)
        ).astype(BF16)                               # [128, HT, 640]
        wvT = np.ascontiguousarray(
            vr.T.reshape(HT, 128, 128).transpose(1, 0, 2)
        ).astype(BF16)
        woT = np.ascontiguousarray(
            W_o[512 * g: 512 * (g + 1)].T.reshape(HT, 128, 512).transpose(1, 0, 2)
        ).astype(BF16)
        in_maps.append({
            "xT": xTb[b], "wqk": wqkT, "wv": wvT, "wo": woT,
            "cc": cc, "ss": ss, "msk": mask, "ones128": ones128,
        })
    return in_maps


def kernel(x, W_qkv, W_o):
    global _COMPILED
    if _COMPILED is None:
        _COMPILED = _build()
    nc = _COMPILED
    in_maps = _prep_inputs(x, W_qkv, W_o)
    res = run_bass_kernel_spmd(nc, in_maps, list(range(8)))
    out = np.empty((B, S, HIDDEN), dtype=np.float32)
    for c in range(8):
        b, g = c // 4, c % 4
        oT = res.results[c]["outT"]                  # [512, 2048]
        out[b, :, 512 * g: 512 * (g + 1)] = oT.T
    return out


# revision 19
# speedup vs baseline: 1.1466x; 1.0425x over previous
"""Trainium2 Bass kernel for nn_Attention_4183298146960.

GQA causal attention layer: B=2, S=2048, HIDDEN=2048, 16 q heads / 4 kv heads,
head_dim=128, RoPE (interleaved pairs), causal softmax, output projection.

Sharding (8 cores, SPMD-uniform program, batch-split tensor parallel):
  core c owns batch b = c//4 and q heads {4g..4g+3}, kv head g, where g = c%4.
  QKV + RoPE + attention fully local (no duplicated kv compute across cores).
  The output projection needs all 16 heads' features, so per-core attention
  outputs [512 feat, 512 tok] are AllGathered within the 4-core batch group
  ([[0,1,2,3],[4,5,6,7]]); each core then computes its 512 output columns.

Schedule: attention chunk qt runs immediately after t-tile qt's QKV (causal:
  it only needs k/v tiles <= qt), so the first AllGather triggers ~45us into
  the kernel and the serialized collective chain hides under the remaining
  compute.  The attention inner loop is ACT(exp)-bound, so W_o matmuls for
  chunks whose gather has landed are interleaved INTO the kt loop (between
  the score and PV matmuls) — the PE chews ready W_o work while exps pend.

Layouts on device (partition dim first):
  feature-major qT/kT/vT [head_dim, tokens]; v is then PE-transposed to
  token-major [tokens, head_dim] for PV (weights for q/k/v all live in one
  lhsT tensor so every projection matmul has a 512-wide free dim).
  Scores are computed transposed [k, q] so softmax needs no max-subtraction
  (scores are O(+-10), exp is fp32-safe) and the denominator is a
  ones-matrix matmul producing the broadcast denominator directly;
  probabilities stay unnormalized until after PV.
  RoPE head dims are permuted [even | odd] via host-side W row permutation
  so the rotation is a 64-partition swap (SBUF->SBUF DMA) + DVE ops.
"""

from collections import deque

import numpy as np
import ml_dtypes

import concourse.bass as bass
import concourse.mybir as mybir
import concourse.tile as tile
from concourse import bacc
from concourse.bass_utils import run_bass_kernel_spmd

BF16 = ml_dtypes.bfloat16

HEADS = 16
KV_HEADS = 4
HIDDEN = 2048
HD = 128
S = 2048
B = 2
HT = HIDDEN // 128             # 16 hidden tiles
NH = 4                         # local q heads per core
SCALE = 1.0 / float(np.sqrt(HD))
RG = [[0, 1, 2, 3], [4, 5, 6, 7]]

_COMPILED = None


def _build():
    dt = mybir.dt
    nc = bacc.Bacc("TRN2", target_bir_lowering=False, debug=False, num_devices=8)

    xT = nc.dram_tensor("xT", [128, HT, S], dt.bfloat16, kind="ExternalInput")
    wqk = nc.dram_tensor("wqk", [128, HT, 768], dt.bfloat16, kind="ExternalInput")
    wo = nc.dram_tensor("wo", [128, HT, 512], dt.bfloat16, kind="ExternalInput")
    cc = nc.dram_tensor("cc", [128, S], dt.bfloat16, kind="ExternalInput")
    ss = nc.dram_tensor("ss", [128, S], dt.bfloat16, kind="ExternalInput")
    msk = nc.dram_tensor("msk", [128, 4 * 512], dt.bfloat16, kind="ExternalInput")
    ones128 = nc.dram_tensor("ones128", [128, 128], dt.bfloat16, kind="ExternalInput")
    ident = nc.dram_tensor("ident", [128, 128], dt.bfloat16, kind="ExternalInput")
    outT = nc.dram_tensor("outT", [512, S], dt.float32, kind="ExternalOutput")

    mult = mybir.AluOpType.mult
    add = mybir.AluOpType.add
    Exp = mybir.ActivationFunctionType.Exp

    with tile.TileContext(nc) as tc:
        with (
            tc.tile_pool(name="const", bufs=1) as constp,
            tc.tile_pool(name="dram", bufs=1, space="DRAM") as dram,
            tc.tile_pool(name="xp", bufs=2) as xp,
            tc.tile_pool(name="rp", bufs=4) as rp,
            tc.tile_pool(name="probs", bufs=6) as probs,
            tc.tile_pool(name="smallp", bufs=4) as smallp,
            tc.tile_pool(name="ap", bufs=3) as apool,
            tc.tile_pool(name="wosb", bufs=2) as wosb,
            tc.tile_pool(name="outp", bufs=2) as outp,
            tc.tile_pool(name="qkps", bufs=2, space="PSUM") as qkps,
            tc.tile_pool(name="spool", bufs=4, space="PSUM") as spool,
            tc.tile_pool(name="pvp", bufs=1, space="PSUM") as pvp,
        ):
            qcat = constp.tile([128, NH * S], dt.bfloat16)  # 4 local q heads
            kT = constp.tile([128, S], dt.bfloat16)
            vsb = constp.tile([128, S], dt.bfloat16)        # token-major v tiles
            wo_sb = constp.tile([128, HT, 512], dt.bfloat16)
            msk_sb = constp.tile([128, 4 * 512], dt.bfloat16)
            ones_sb = constp.tile([128, 128], dt.bfloat16)
            ident_sb = constp.tile([128, 128], dt.bfloat16)
            wqk_sb = constp.tile([128, HT, 768], dt.bfloat16)
            cc_sb = constp.tile([128, S], dt.bfloat16)
            ss_sb = constp.tile([128, S], dt.bfloat16)

            # first quarter of the qkv weights only — x(t0) must follow quickly
            nc.sync.dma_start(wqk_sb[:, 0:4, :], wqk[:, 0:4, :])

            class WoFiller:
                """Doles out W_o matmuls one at a time into attention gaps."""

                def __init__(self):
                    self.gens = deque()

                def _gen(self, qt, ag_out):
                    asb = wosb.tile([128, HT, 512], dt.bfloat16)
                    for dtt in range(HT):
                        nc.sync.dma_start(asb[:, dtt, :], ag_out[dtt, :, :])
                    for ct in range(4):
                        ps_o = qkps.tile([128, 512], dt.float32, name="qk")
                        for dtt in range(HT):
                            nc.tensor.matmul(
                                ps_o[:],
                                lhsT=wo_sb[:, dtt, ct * 128:(ct + 1) * 128],
                                rhs=asb[:, dtt, :],
                                start=(dtt == 0), stop=(dtt == HT - 1),
                            )
                            yield
                        o_sb = outp.tile([128, 512], dt.float32)
                        nc.vector.tensor_copy(o_sb[:], ps_o[:])
                        nc.sync.dma_start(
                            outT[ct * 128:(ct + 1) * 128, qt * 512:(qt + 1) * 512],
                            o_sb[:],
                        )

                def add(self, qt, ag_out):
                    self.gens.append(self._gen(qt, ag_out))

                def step(self, n):
                    while n > 0 and self.gens:
                        try:
                            next(self.gens[0])
                            n -= 1
                        except StopIteration:
                            self.gens.popleft()

                def flush(self):
                    while self.gens:
                        self.step(64)

            woq = WoFiller()

            def emit_att(qt, j):
                """Attention for 512-token chunk qt (k/v tiles <= 4qt+3 ready)."""
                attn_chunk = dram.tile([512, 512], dt.bfloat16, name=f"attnc{j}")
                kts = 4 * qt + 4
                for hp in range(2):            # head pairs (2hp, 2hp+1)
                    ps_pv = [pvp.tile([128, 512], dt.float32, name=f"pv{hh}")
                             for hh in range(2)]
                    acc = [smallp.tile([128, 512], dt.bfloat16, name=f"acc{hh}", bufs=2)
                           for hh in range(2)]
                    for kt in range(kts):
                        r = kt - 4 * qt
                        prob2 = []
                        for hh in range(2):
                            h = 2 * hp + hh
                            ps_s = spool.tile([128, 512], dt.float32)
                            nc.tensor.matmul(
                                ps_s[:],
                                lhsT=kT[:, kt * 128:(kt + 1) * 128],
                                rhs=qcat[:, h * S + qt * 512: h * S + (qt + 1) * 512],
                                start=True,
                                stop=True,
                            )
                            prob = probs.tile([128, 512], dt.bfloat16)
                            if r >= 0:
                                stg = probs.tile([128, 512], dt.bfloat16, name="stg")
                                nc.scalar.activation(stg[:], ps_s[:], Exp, scale=SCALE)
                                nc.vector.tensor_tensor(
                                    prob[:], stg[:], msk_sb[:, r * 512:(r + 1) * 512], mult
                                )
                            else:
                                nc.scalar.activation(prob[:], ps_s[:], Exp, scale=SCALE)
                            prob2.append(prob)
                        # ready W_o work fills the PE while the exps run
                        woq.step(4)
                        for hh in range(2):
                            nc.tensor.matmul(
                                ps_pv[hh][:],
                                lhsT=vsb[:, kt * 128:(kt + 1) * 128],
                                rhs=prob2[hh][:],
                                start=(kt == 0), stop=(kt == kts - 1),
                            )
                        for hh in range(2):
                            # denominator partial sums on DVE (sum over k-tiles)
                            if kt == 0:
                                nc.vector.tensor_copy(acc[hh][:], prob2[hh][:])
                            else:
                                nc.vector.tensor_add(acc[hh][:], acc[hh][:], prob2[hh][:])
                    for hh in range(2):
                        # partition-reduce + broadcast denominators in one matmul
                        ps_den = qkps.tile([128, 512], dt.float32, name="qk")
                        nc.tensor.matmul(
                            ps_den[:], lhsT=ones_sb[:], rhs=acc[hh][:],
                            start=True, stop=True,
                        )
                        den_sb = smallp.tile([128, 512], dt.float32)
                        nc.vector.reciprocal_approx_fast(den_sb[:], ps_den[:])
                        attn_sb = apool.tile([128, 512], dt.bfloat16)
                        nc.vector.tensor_tensor(attn_sb[:], ps_pv[hh][:], den_sb[:], mult)
                        lh = 2 * hp + hh
                        nc.sync.dma_start(
                            attn_chunk[lh * 128:(lh + 1) * 128, :], attn_sb[:]
                        )
                        woq.step(4)
                ag_out = dram.tile([HT, 128, 512], dt.bfloat16, name=f"agout{j}")
                nc.gpsimd.collective_compute(
                    "AllGather", mybir.AluOpType.bypass, replica_groups=RG,
                    ins=[attn_chunk.opt()], outs=[ag_out.opt()],
                )
                return ag_out

            pending_wo = []
            for tt in range(4):
                x_sb = xp.tile([128, HT, 512], dt.bfloat16)
                for hq in range(4):  # split so MMs can start on early ht tiles
                    nc.sync.dma_start(
                        x_sb[:, hq * 4:(hq + 1) * 4, :],
                        xT[:, hq * 4:(hq + 1) * 4, tt * 512:(tt + 1) * 512],
                    )
                if tt == 0:
                    # remaining weights/tables, behind the x(t0) stream
                    nc.sync.dma_start(cc_sb[:], cc[:])
                    nc.sync.dma_start(ss_sb[:], ss[:])
                    for hq in range(1, 4):
                        nc.sync.dma_start(
                            wqk_sb[:, hq * 4:(hq + 1) * 4, :],
                            wqk[:, hq * 4:(hq + 1) * 4, :],
                        )
                    nc.sync.dma_start(msk_sb[:], msk[:])
                    nc.sync.dma_start(ones_sb[:], ones128[:])
                    nc.sync.dma_start(ident_sb[:], ident[:])
                    nc.sync.dma_start(wo_sb[:], wo[:])
                tsl = bass.ts(tt, 512)
                for ft in range(6):            # q0..q3, k, v
                    ps = qkps.tile([128, 512], dt.float32, name="qk")
                    for ht in range(HT):
                        nc.tensor.matmul(
                            ps[:],
                            lhsT=wqk_sb[:, ht, ft * 128:(ft + 1) * 128],
                            rhs=x_sb[:, ht, :],
                            start=(ht == 0),
                            stop=(ht == HT - 1),
                        )
                    sbq = rp.tile([128, 512], dt.bfloat16)
                    nc.scalar.copy(sbq[:], ps[:])
                    if ft == 5:
                        # v: feature-major [hd, tok]; PE-transpose to token-major
                        # (bf16 psum tile: transpose output dtype must match input)
                        ps_t = qkps.tile([128, 1024], dt.bfloat16, name="qk")
                        for st in range(4):
                            nc.tensor.transpose(
                                ps_t[:, st * 128:(st + 1) * 128],
                                sbq[:, st * 128:(st + 1) * 128],
                                ident_sb[:],
                            )
                        nc.scalar.copy(vsb[:, tsl], ps_t[:, 0:512])
                        continue
                    tmp = rp.tile([128, 512], dt.bfloat16)
                    # sync queue, NOT gpsimd: the collective trigger+wait
                    # occupies the gpsimd queue and would gate the rope here
                    nc.sync.dma_start(tmp[0:64, :], sbq[64:128, :])
                    nc.sync.dma_start(tmp[64:128, :], sbq[0:64, :])
                    qcc = rp.tile([128, 512], dt.bfloat16)
                    nc.vector.tensor_tensor(qcc[:], sbq[:], cc_sb[:, tsl], mult)
                    qss = rp.tile([128, 512], dt.bfloat16)
                    nc.vector.tensor_tensor(qss[:], tmp[:], ss_sb[:, tsl], mult)
                    if ft < 4:
                        dst = qcat[:, ft * S + tt * 512: ft * S + (tt + 1) * 512]
                    else:
                        dst = kT[:, tsl]
                    nc.vector.tensor_tensor(dst, qcc[:], qss[:], add)
                # W_o for gathered chunks interleaves into this chunk's attention
                while len(pending_wo) >= 2:
                    woq.add(*pending_wo.pop(0))
                ag_out = emit_att(tt, tt)
                pending_wo.append((tt, ag_out))
            woq.flush()
            for qt, ag_out in pending_wo:
                woq.add(qt, ag_out)
                woq.flush()
    nc.compile()
    return nc


# host-side input prep ------------------------------------------------------

_PERM = np.concatenate([np.arange(0, HD, 2), np.arange(1, HD, 2)])


def _rope_tables():
    freq = 1.0 / (10000.0 ** (np.arange(0, HD, 2, dtype=np.float64) / HD))
    pos = np.arange(S, dtype=np.float64)
    ang = np.outer(pos, freq)                       # [S, 64]
    cos = np.cos(ang).T.astype(np.float32)          # [64, S]
    sin = np.sin(ang).T.astype(np.float32)
    cc1 = np.concatenate([cos, cos], 0)             # [128, S]
    ss1 = np.concatenate([-sin, sin], 0)            # [128, S]
    return cc1.astype(BF16), ss1.astype(BF16)


def _prep_inputs(x, W_qkv, W_o):
    x = np.asarray(x, dtype=np.float32)
    W_qkv = np.asarray(W_qkv, dtype=np.float32)
    W_o = np.asarray(W_o, dtype=np.float32)

    xTb = []
    for b in range(B):
        xTb.append(np.ascontiguousarray(
            x[b].T.reshape(HT, 128, S).transpose(1, 0, 2)
        ).astype(BF16))                              # [128, HT, 2048]

    cc, ss = _rope_tables()

    mask = np.zeros((128, 4 * 512), dtype=np.float32)
    ii = np.arange(128)[:, None]
    jj = np.arange(512)[None, :]
    for r in range(4):
        mask[:, r * 512:(r + 1) * 512] = (jj >= ii + 128 * r)
    mask = mask.astype(BF16)

    ones128 = np.ones((128, 128), dtype=np.float32).astype(BF16)
    ident = np.eye(128, dtype=np.float32).astype(BF16)

    in_maps = []
    for c in range(8):
        b, g = c // 4, c % 4
        qr = W_qkv[512 * g: 512 * (g + 1)]           # rows of q heads 4g..4g+3
        qr = qr.reshape(NH, HD, HIDDEN)[:, _PERM, :].reshape(512, HIDDEN)
        kr = W_qkv[HIDDEN + 128 * g: HIDDEN + 128 * (g + 1)][_PERM, :]
        vr = W_qkv[HIDDEN + 512 + 128 * g: HIDDEN + 512 + 128 * (g + 1)]
        wqkT = np.ascontiguousarray(
            np.concatenate([qr, kr, vr], 0).T.reshape(HT, 128, 768).transpose(1, 0, 2)
        ).astype(BF16)                               # [128, HT, 768]
        woT = np.ascontiguousarray(
            W_o[512 * g: 512 * (g + 1)].T.reshape(HT, 128, 512).transpose(1, 0, 2)
        ).astype(BF16)
        in_maps.append({
            "xT": xTb[b], "wqk": wqkT, "wo": woT,
            "cc": cc, "ss": ss, "msk": mask, "ones128": ones128, "ident": ident,
        })
    return in_maps


def kernel(x, W_qkv, W_o):
    global _COMPILED
    if _COMPILED is None:
        _COMPILED = _build()
    nc = _COMPILED
    in_maps = _prep_inputs(x, W_qkv, W_o)
    res = run_bass_kernel_spmd(nc, in_maps, list(range(8)))
    out = np.empty((B, S, HIDDEN), dtype=np.float32)
    for c in range(8):
        b, g = c // 4, c % 4
        oT = res.results[c]["outT"]                  # [512, 2048]
        out[b, :, 512 * g: 512 * (g + 1)] = oT.T
    return out
